# revision 1
# baseline (speedup 1.0000x reference)
"""Causal multi-head attention (B=2, T=2048, E=1024, 16 heads) on 8 TRN2 cores.

Sharding: 8-way tensor-parallel over heads (2 heads/core) for QKV projections
and attention; one AllToAll per head re-shards the attention output over
tokens so each core computes the output projection for its 512-token block.

All matmuls run in float32r (TF32-like, full PE rate at free-dim >= 256).
The host passes x^T and the weight transposes directly as float32r inputs,
so no on-device transposes are needed except for V (computed on device).
Scores are computed transposed (S^T = K Q^T, [k-toks x q-toks]) so softmax
P^T feeds the AV matmul directly; a ones column appended to V makes the AV
matmul emit softmax denominators; causal masking is one 128x128 triangle
add per diagonal block plus block-level skipping; max-subtraction is
omitted (scores are O(1), exp cannot overflow).
"""
import sys

if "/opt/trn_rl_repo" not in sys.path:
    sys.path.insert(0, "/opt/trn_rl_repo")

import numpy as np

import concourse.bacc as bacc
import concourse.mybir as mybir
from concourse import tile
from concourse.bass_utils import run_bass_kernel_spmd

dt = mybir.dt
AF = mybir.ActivationFunctionType
ALU = mybir.AluOpType

B, T, E, HS, NH = 2, 2048, 1024, 64, 16
NCORE = 8
NTOK = B * T            # 4096
CH = 512                # token chunk
NCH = NTOK // CH        # 8
CPB = NCH // B          # chunks per batch = 4
SUB = 128
NSUB = CH // SUB        # 4
NEG = -1.0e30

_nc_cache = {}


def build_nc(stage="full"):
    nc = bacc.Bacc("TRN2", target_bir_lowering=False, debug=False,
                   num_devices=NCORE)
    f32, f32r = dt.float32, dt.float32r

    xT = nc.declare_dram_parameter("xT", [E, NTOK], f32r, isOutput=False)
    wqT = nc.declare_dram_parameter("wqT", [E, 128], f32r, isOutput=False)
    wkT = nc.declare_dram_parameter("wkT", [E, 128], f32r, isOutput=False)
    wvT = nc.declare_dram_parameter("wvT", [E, 128], f32r, isOutput=False)
    woh0 = nc.declare_dram_parameter("woh0", [512, E], f32r, isOutput=False)
    woh1 = nc.declare_dram_parameter("woh1", [512, E], f32r, isOutput=False)
    bqs = nc.declare_dram_parameter("bqs", [128, 1], f32, isOutput=False)
    bks = nc.declare_dram_parameter("bks", [128, 1], f32, isOutput=False)
    bvs = nc.declare_dram_parameter("bvs", [128, 1], f32, isOutput=False)
    bo_b = nc.declare_dram_parameter("bo_b", [128, E], f32, isOutput=False)
    eye = nc.declare_dram_parameter("eye", [128, 128], f32, isOutput=False)
    tri = nc.declare_dram_parameter("tri", [128, 128], f32, isOutput=False)
    ones_v = nc.declare_dram_parameter("ones_v", [128, NCH * NSUB], f32,
                                       isOutput=False)
    ones_r = nc.declare_dram_parameter("ones_r", [1, 64], f32, isOutput=False)
    y = nc.declare_dram_parameter("y", [CH, E], f32, isOutput=True)

    with tile.TileContext(nc) as tc:
        from contextlib import ExitStack
        with ExitStack() as top:
            const = top.enter_context(tc.tile_pool(name="const", bufs=1))
            persist = top.enter_context(tc.tile_pool(name="persist", bufs=1))
            xtp_pool = top.enter_context(tc.tile_pool(name="xtp", bufs=2))
            ps_t = top.enter_context(
                tc.tile_pool(name="ps_t", bufs=1, space="PSUM"))
            ps_q = top.enter_context(
                tc.tile_pool(name="ps_q", bufs=2, space="PSUM"))
            ps_s = top.enter_context(
                tc.tile_pool(name="ps_s", bufs=3, space="PSUM"))
            ps_a = top.enter_context(
                tc.tile_pool(name="ps_a", bufs=2, space="PSUM"))
            dram = top.enter_context(
                tc.tile_pool(name="dram", bufs=1, space="DRAM"))

            # ---- constants -------------------------------------------------
            eye_sb = const.tile([128, 128], f32, name="eye_sb")
            nc.sync.dma_start(eye_sb[:], eye[:])
            eyer_sb = const.tile([128, 128], f32r, name="eyer_sb")
            nc.vector.tensor_copy(eyer_sb[:], eye_sb[:])
            tri_sb = const.tile([128, 128], f32, name="tri_sb")
            nc.sync.dma_start(tri_sb[:], tri[:])
            onesv_sb = const.tile([128, NCH * NSUB], f32, name="onesv_sb")
            nc.sync.dma_start(onesv_sb[:], ones_v[:])
            onesr_sb = const.tile([1, 64], f32, name="onesr_sb")
            nc.sync.dma_start(onesr_sb[:], ones_r[:])
            onesr_r = const.tile([1, 64], f32r, name="onesr_r")
            nc.vector.tensor_copy(onesr_r[:], onesr_sb[:])
            bq_sb = const.tile([128, 1], f32, name="bq_sb")
            nc.sync.dma_start(bq_sb[:], bqs[:])
            bk_sb = const.tile([128, 1], f32, name="bk_sb")
            nc.sync.dma_start(bk_sb[:], bks[:])
            bv_sb = const.tile([128, 1], f32, name="bv_sb")
            nc.sync.dma_start(bv_sb[:], bvs[:])
            bo_sb = const.tile([128, E], f32, name="bo_sb")
            nc.sync.dma_start(bo_sb[:], bo_b[:])

            # ---- persistent tensors ---------------------------------------
            wq_sb = persist.tile([128, 8, 128], f32r, name="wq_sb")
            wk_sb = persist.tile([128, 8, 128], f32r, name="wk_sb")
            wv_sb = persist.tile([128, 8, 128], f32r, name="wv_sb")
            wo0_sb = persist.tile([128, 4, E], f32r, name="wo0_sb")
            wo1_sb = persist.tile([128, 4, E], f32r, name="wo1_sb")
            nc.sync.dma_start(wq_sb[:], wqT.rearrange("(e p) m -> p e m", p=128))
            nc.sync.dma_start(wk_sb[:], wkT.rearrange("(e p) m -> p e m", p=128))
            nc.sync.dma_start(wv_sb[:], wvT.rearrange("(e p) m -> p e m", p=128))
            kT = persist.tile([128, NCH, CH], f32r, name="kT")
            qT = persist.tile([128, NCH, CH], f32r, name="qT")
            vh0 = persist.tile([128, NCH * NSUB, 65], f32r, name="vh0")
            vh1 = persist.tile([128, NCH * NSUB, 65], f32r, name="vh1")

            cc_in = [dram.tile([NCH, 64, CH], f32r, name=f"cc_in{h}")
                     for h in range(2)]
            cc_out = [dram.tile([NCH, 64, CH], f32r, name=f"cc_out{h}")
                      for h in range(2)]

            # ones column of the augmented V
            nc.vector.tensor_copy(vh0[:, :, 64], onesv_sb[:])
            nc.vector.tensor_copy(vh1[:, :, 64], onesv_sb[:])

            # ---- phases B+C interleaved: QKV chunk t, then attention t-1 ---
            # B's dense QKV matmul bursts fill the PE between C's
            # scores->exp->AV chains, keeping the HAM clock warm.
            vstage = top.enter_context(tc.tile_pool(name="vstage", bufs=2))
            ppool = top.enter_context(tc.tile_pool(name="ppool", bufs=5))
            apool = top.enter_context(tc.tile_pool(name="apool", bufs=2))

            def emit_b(t):
                xTt = xtp_pool.tile([128, 8, CH], f32r, name="xTt",
                                    tag="xTt")
                for e in range(8):
                    for half in range(2):
                        nc.sync.dma_start(
                            xTt[:, e, 256 * half:256 * (half + 1)],
                            xT[128 * e:128 * (e + 1),
                               CH * t + 256 * half:CH * t + 256 * (half + 1)])

                # Q^T (scale 1/8 folded), K^T
                for wsb, bias, scale, dest in (
                        (wq_sb, bq_sb, 0.125, qT),
                        (wk_sb, bk_sb, 1.0, kT)):
                    ps = ps_q.tile([128, CH], f32, name="psqk", tag="psq")
                    for e in range(8):
                        nc.tensor.matmul(ps[:], wsb[:, e, :], xTt[:, e, :],
                                         start=(e == 0), stop=(e == 7))
                    if scale == 1.0:
                        nc.vector.tensor_scalar_add(dest[:, t, :], ps[:],
                                                    bias[:])
                    else:
                        nc.vector.tensor_scalar(
                            dest[:, t, :], ps[:], scale, bias[:],
                            ALU.mult, ALU.add)

                # V^T then transpose to V rows, split per head
                psv = ps_q.tile([128, CH], f32, name="psv", tag="psq")
                for e in range(8):
                    nc.tensor.matmul(psv[:], wv_sb[:, e, :], xTt[:, e, :],
                                     start=(e == 0), stop=(e == 7))
                vTs = vstage.tile([128, CH], f32r, name="vTs", tag="vTs")
                nc.vector.tensor_scalar_add(vTs[:], psv[:], bv_sb[:])
                for s in range(NSUB):
                    tv = ps_q.tile([128, 512], f32r, name="tpv", tag="psq")
                    nc.tensor.transpose(
                        tv[:, 0:128], vTs[:, 128 * s:128 * (s + 1)],
                        eyer_sb[:])
                    g = NSUB * t + s
                    nc.vector.tensor_copy(vh0[:, g, 0:64], tv[:, 0:64])
                    nc.vector.tensor_copy(vh1[:, g, 0:64], tv[:, 64:128])

            def emit_c(t):
                b0 = CPB * (t // CPB)
                a_pss = [ps_a.tile([128, CH], f32, name=f"a_ps{h}",
                                   tag="aps") for h in range(2)]

                def emit_scores(h, kc):
                    pb = 64 * h
                    diag = kc == t
                    pT = ppool.tile([128, NSUB, CH], f32r,
                                    name="pT", tag="pT")
                    for s in range(NSUB):
                        q0 = 128 * s if diag else 0
                        sps = ps_s.tile([128, CH], f32,
                                        name="sps", tag="sps")
                        nc.tensor.matmul(
                            sps[:, q0:CH],
                            kT[pb:pb + 64, kc, 128 * s:128 * (s + 1)],
                            qT[pb:pb + 64, t, q0:CH],
                            start=True, stop=True)
                        if diag:
                            nc.vector.tensor_add(
                                sps[:, q0:q0 + 128],
                                sps[:, q0:q0 + 128], tri_sb[:])
                        nc.scalar.activation(
                            pT[:, s, q0:CH], sps[:, q0:CH], AF.Exp)
                    return pT

                def emit_av(h, kc, pT):
                    vh = vh0 if h == 0 else vh1
                    diag = kc == t
                    for s in range(NSUB):
                        q0 = 128 * s if diag else 0
                        g = NSUB * kc + s
                        nc.tensor.matmul(
                            a_pss[h][0:65, q0:CH], vh[:, g, :],
                            pT[:, s, q0:CH],
                            start=(kc == b0 and s == 0),
                            stop=(diag and s == NSUB - 1))

                prev = None
                for kc in range(b0, t + 1):
                    pTs = (emit_scores(0, kc), emit_scores(1, kc))
                    if prev is not None:
                        emit_av(0, prev[0], prev[1][0])
                        emit_av(1, prev[0], prev[1][1])
                    prev = (kc, pTs)
                emit_av(0, prev[0], prev[1][0])
                emit_av(1, prev[0], prev[1][1])

                for h in range(2):
                    rec = apool.tile([1, CH], f32r, name="rec", tag="rec")
                    with nc.allow_low_precision(
                            reason="f32r recip feeds PE broadcast; "
                                   "psum accum stays fp32"):
                        nc.vector.reciprocal(rec[:], a_pss[h][64:65, :])
                    bc_ps = ps_t.tile([64, CH], f32, name="bc_ps",
                                      tag="pst", bufs=1)
                    nc.tensor.matmul(bc_ps[:], onesr_r[:], rec[:],
                                     start=True, stop=True)
                    bc_sb = apool.tile([64, CH], f32r, name="bc_sb",
                                       tag="bcs")
                    nc.vector.tensor_copy(bc_sb[:], bc_ps[:])
                    a_sb = apool.tile([64, CH], f32r, name="a_sb",
                                      tag="asb")
                    nc.vector.tensor_mul(a_sb[:], a_pss[h][0:64, :],
                                         bc_sb[:])
                    nc.sync.dma_start(cc_in[h][t, :, :], a_sb[:])

            for t in range(NCH):
                emit_b(t)
                if stage != "qkv" and t >= 1:
                    emit_c(t - 1)
            for r in range(4):
                nc.sync.dma_start(wo0_sb[:, r, :],
                                  woh0[128 * r:128 * (r + 1), :])
                nc.sync.dma_start(wo1_sb[:, r, :],
                                  woh1[128 * r:128 * (r + 1), :])

            if stage != "qkv":
                emit_c(NCH - 1)
                for h in range(2):
                    nc.gpsimd.collective_compute(
                        "AllToAll", ALU.bypass,
                        ins=[cc_in[h].opt()], outs=[cc_out[h].opt()],
                        replica_groups=[list(range(NCORE))])

            if stage == "qkv":
                yv = y.rearrange("(s p) e -> p s e", p=128)
                dbg = persist.tile([128, 8, CH], f32, name="dbg")
                nc.vector.tensor_copy(dbg[:], qT[:].bitcast(f32))
                nc.sync.dma_start(yv, dbg.rearrange("p c t -> p (c t)").rearrange("p (s e) -> p s e", s=4))

            if stage == "attn":
                yv = y.rearrange("(s p) e -> p s e", p=128)
                for h in range(2):
                    for c in range(NCH):
                        nc.sync.dma_start(
                            yv[64 * h:64 * (h + 1), c // 2,
                               (c % 2) * 512:(c % 2) * 512 + 512],
                            cc_in[h][c, :, :].bitcast(f32))

            # ---- phase E: output projection on this core's token block -----
            # split by head: the h0 half runs as soon as A2A#0 lands and
            # overlaps A2A#1; the h1 half adds the h0 partial and stores.
            with tc.tile_pool(name="ystage", bufs=2) as ystage:
                if stage == "full":
                    yacc = xtp_pool.tile([128, NSUB, E], f32, name="yacc",
                                         tag="xTt")
                    aTb = xtp_pool.tile([128, 2, 4, CH], f32r, name="aTb",
                                        tag="xTt")
                    aTs = [aTb[:, 0], aTb[:, 1]]
                    for h, cco in enumerate(cc_out):
                        for kt in range(8):
                            nc.sync.dma_start(
                                aTb[64 * (kt % 2):64 * (kt % 2) + 64,
                                    h, kt // 2, :],
                                cco[kt, :, :])
                    for m in range(NSUB):
                        for nch in range(2):
                            yps = ps_t.tile([128, 512], f32, name="yps",
                                            tag="pst", bufs=1)
                            for p in range(4):
                                nc.tensor.matmul(
                                    yps[:],
                                    aTs[0][:, p, 128 * m:128 * (m + 1)],
                                    wo0_sb[:, p, 512 * nch:512 * (nch + 1)],
                                    start=(p == 0), stop=(p == 3))
                            nc.vector.tensor_add(
                                yacc[:, m, 512 * nch:512 * (nch + 1)], yps[:],
                                bo_sb[:, 512 * nch:512 * (nch + 1)])
                for m in (range(NSUB) if stage == "full" else []):
                    for nch in range(2):
                        yps = ps_t.tile([128, 512], f32, name="yps",
                                        tag="pst", bufs=1)
                        for p in range(4):
                            nc.tensor.matmul(
                                yps[:], aTs[1][:, p, 128 * m:128 * (m + 1)],
                                wo1_sb[:, p, 512 * nch:512 * (nch + 1)],
                                start=(p == 0), stop=(p == 3))
                        ysb = ystage.tile([128, 512], f32, name="ysb",
                                          tag="ysb")
                        nc.vector.tensor_add(
                            ysb[:], yps[:],
                            yacc[:, m, 512 * nch:512 * (nch + 1)])
                        nc.sync.dma_start(
                            y[128 * m:128 * (m + 1),
                              512 * nch:512 * (nch + 1)],
                            ysb[:])
    nc.compile()
    return nc


def _prep_in_maps(embd_q, Wq, bq, Wk, bk, Wv, bv, Wo, bo):
    x = embd_q.reshape(NTOK, E).astype(np.float32)
    xT = np.ascontiguousarray(x.T)
    eye = np.eye(128, dtype=np.float32)
    r = np.arange(128)
    tri = np.where(r[:, None] > r[None, :], np.float32(NEG), np.float32(0.0))
    tri = np.ascontiguousarray(tri, dtype=np.float32)
    ones_v = np.ones((128, NCH * NSUB), dtype=np.float32)
    ones_r = np.ones((1, 64), dtype=np.float32)
    bo_b = np.ascontiguousarray(
        np.broadcast_to(bo.astype(np.float32), (128, E)))
    woTf = Wo.astype(np.float32).T  # [feat, out]
    # pair-interleaved per-head layouts: partition q of pair p maps to
    # feat = 128*(2p) + q  (q < 64, even kt)  or  128*(2p+1) + (q-64)
    idx = np.zeros((4, 128), dtype=np.int64)
    for p in range(4):
        idx[p, :64] = 128 * (2 * p) + np.arange(64)
        idx[p, 64:] = 128 * (2 * p + 1) + np.arange(64)
    woh0 = np.ascontiguousarray(woTf[idx.reshape(-1)])
    woh1 = np.ascontiguousarray(woTf[(idx + 64).reshape(-1)])
    in_maps = []
    for c in range(NCORE):
        sl = slice(128 * c, 128 * (c + 1))
        in_maps.append({
            "xT": xT,
            "wqT": np.ascontiguousarray(Wq[sl].astype(np.float32).T),
            "wkT": np.ascontiguousarray(Wk[sl].astype(np.float32).T),
            "wvT": np.ascontiguousarray(Wv[sl].astype(np.float32).T),
            "woh0": woh0,
            "woh1": woh1,
            "bqs": np.ascontiguousarray(
                (bq[sl] * 0.125).reshape(128, 1), dtype=np.float32),
            "bks": np.ascontiguousarray(bk[sl].reshape(128, 1),
                                        dtype=np.float32),
            "bvs": np.ascontiguousarray(bv[sl].reshape(128, 1),
                                        dtype=np.float32),
            "bo_b": bo_b,
            "eye": eye,
            "tri": tri,
            "ones_v": ones_v,
            "ones_r": ones_r,
        })
    return in_maps


def kernel(embd_q, Wq, bq, Wk, bk, Wv, bv, Wo, bo, _trace=False,
           _stage="full"):
    if _stage not in _nc_cache:
        _nc_cache[_stage] = build_nc(_stage)
    in_maps = _prep_in_maps(np.asarray(embd_q), np.asarray(Wq), np.asarray(bq),
                            np.asarray(Wk), np.asarray(bk), np.asarray(Wv),
                            np.asarray(bv), np.asarray(Wo), np.asarray(bo))
    import os
    tc_env = os.environ.get("TRACE_CORES")
    res = run_bass_kernel_spmd(
        _nc_cache[_stage], in_maps, list(range(NCORE)), trace=_trace,
        trace_cores=(list(range(NCORE)) if tc_env else None))
    out = np.concatenate(
        [res.results[c]["y"] for c in range(NCORE)], axis=0)
    out = out.reshape(B, T, E)
    kernel.last_results = res
    return out



# revision 43
# speedup vs baseline: 1.3687x; 1.3687x over previous
"""Causal multi-head attention (B=2, T=2048, E=1024, 16 heads) on 8 TRN2 cores.

Sharding: 8-way tensor-parallel over heads (2 heads/core) for QKV projections
and attention; one AllToAll re-shards the attention output over tokens so
each core computes the output projection for its 512-token block.

v2: all matmul operands in bf16 (f32 PSUM accumulation), phased schedule
(QKV -> attention -> A2A -> out-proj) to keep the PE HAM clock at 2.4 GHz,
causal mask applied as a PE accumulate-matmul (step x -1e30*I) instead of
DVE triangle adds, softmax denominators via reciprocal_approx_fast, exp
batched two k-subblocks per ACT instruction, single bf16 AllToAll.
"""
import sys

if "/opt/trn_rl_repo" not in sys.path:
    sys.path.insert(0, "/opt/trn_rl_repo")

import numpy as np
import ml_dtypes

import concourse.bacc as bacc
import concourse.mybir as mybir
from concourse import tile
from concourse.bass_utils import run_bass_kernel_spmd

dt = mybir.dt
AF = mybir.ActivationFunctionType
ALU = mybir.AluOpType

B, T, E, HS, NH = 2, 2048, 1024, 64, 16
NCORE = 8
NTOK = B * T            # 4096
CH = 512                # token chunk (one chunk per core for out-proj)
NCH = NTOK // CH        # 8
CPB = NCH // B          # chunks per batch = 4
SUB = 128
NSUB = CH // SUB        # 4
NEG = -1.0e30

_nc_cache = {}


def build_nc(stage="full"):
    nc = bacc.Bacc("TRN2", target_bir_lowering=False, debug=False,
                   num_devices=NCORE)
    f32, f32r, bf16 = dt.float32, dt.float32r, dt.bfloat16

    xT = nc.declare_dram_parameter("xT", [E, NTOK], bf16, isOutput=False)
    wqT = nc.declare_dram_parameter("wqT", [E, 128], bf16, isOutput=False)
    wkT = nc.declare_dram_parameter("wkT", [E, 128], bf16, isOutput=False)
    wvT = nc.declare_dram_parameter("wvT", [E, 128], bf16, isOutput=False)
    woT = nc.declare_dram_parameter("woT", [E, E], bf16, isOutput=False)
    bqs = nc.declare_dram_parameter("bqs", [128, 1], f32, isOutput=False)
    bks = nc.declare_dram_parameter("bks", [128, 1], f32, isOutput=False)
    bvs = nc.declare_dram_parameter("bvs", [128, 1], f32, isOutput=False)
    bo_b = nc.declare_dram_parameter("bo_b", [128, E], f32, isOutput=False)
    eye = nc.declare_dram_parameter("eye", [128, 128], f32r, isOutput=False)
    ones_c = nc.declare_dram_parameter("ones_c", [128, NCH * NSUB], bf16,
                                       isOutput=False)
    # causal mask factors: tri(k,q) = -1e30 for k > q is stepU.T @ negI
    stepu = nc.declare_dram_parameter("stepu", [128, 128], bf16,
                                      isOutput=False)
    negi = nc.declare_dram_parameter("negi", [128, 128], bf16, isOutput=False)
    ones_r = nc.declare_dram_parameter("ones_r", [1, 64], bf16, isOutput=False)
    if stage == "qkv":
        y = nc.declare_dram_parameter("y", [128, NCH * CH], bf16,
                                      isOutput=True)
    elif stage == "vh":
        y = nc.declare_dram_parameter("y", [128, 32 * 130], bf16,
                                      isOutput=True)
    elif stage == "attn":
        y = nc.declare_dram_parameter("y", [NCH, 2, 64, CH], bf16,
                                      isOutput=True)
    else:
        y = nc.declare_dram_parameter("y", [CH, E], f32, isOutput=True)

    with tile.TileContext(nc) as tc:
        from contextlib import ExitStack
        with ExitStack() as top:
            const = top.enter_context(tc.tile_pool(name="const", bufs=1))
            persist = top.enter_context(tc.tile_pool(name="persist", bufs=1))
            dram = top.enter_context(
                tc.tile_pool(name="dram", bufs=1, space="DRAM"))

            # ---- constants (stage-gated: dead DMA stores break walrus) -----
            need = {
                "qkv": {"q"},
                "vh": {"v", "eye", "onesc"},
                "attn": {"q", "k", "v", "eye", "onesc", "mask", "norm"},
            }.get(stage, {"q", "k", "v", "eye", "onesc", "mask", "norm", "o"})
            eye_sb = const.tile([128, 128], f32r, name="eye_sb")
            if "eye" in need:
                nc.sync.dma_start(eye_sb[:], eye[:])
            stepu_sb = const.tile([128, 128], bf16, name="stepu_sb")
            negi_sb = const.tile([128, 128], bf16, name="negi_sb")
            if "mask" in need:
                nc.sync.dma_start(stepu_sb[:], stepu[:])
                nc.sync.dma_start(negi_sb[:], negi[:])
            onesr_sb = const.tile([1, 64], bf16, name="onesr_sb")
            if "norm" in need:
                nc.sync.dma_start(onesr_sb[:], ones_r[:])
            bq_sb = const.tile([128, 1], f32, name="bq_sb")
            bk_sb = const.tile([128, 1], f32, name="bk_sb")
            bv_sb = const.tile([128, 1], f32, name="bv_sb")
            bo_sb = const.tile([128, E], f32, name="bo_sb")
            if "q" in need:
                nc.sync.dma_start(bq_sb[:], bqs[:])
            if "k" in need:
                nc.sync.dma_start(bk_sb[:], bks[:])
            if "v" in need:
                nc.sync.dma_start(bv_sb[:], bvs[:])
            if "o" in need:
                nc.sync.dma_start(bo_sb[:], bo_b[:])

            # ---- persistent tensors ---------------------------------------
            wq_sb = persist.tile([128, 8, 128], bf16, name="wq_sb")
            wk_sb = persist.tile([128, 8, 128], bf16, name="wk_sb")
            wv_sb = persist.tile([128, 8, 128], bf16, name="wv_sb")
            if "q" in need:
                nc.sync.dma_start(wq_sb[:],
                                  wqT.rearrange("(e p) m -> p e m", p=128))
            if "k" in need:
                nc.sync.dma_start(wk_sb[:],
                                  wkT.rearrange("(e p) m -> p e m", p=128))
            if "v" in need:
                nc.sync.dma_start(wv_sb[:],
                                  wvT.rearrange("(e p) m -> p e m", p=128))
            wo_sb = persist.tile([128, 8, E], bf16, name="wo_sb")

            # full x^T resident: [128 parts, 8 e-slices, 8 chunks, 512]
            xs = persist.tile([128, 8, NCH, CH], bf16, name="xs")
            # issue chunk-major so chunk 0's slices land first
            for t in range(NCH):
                for e in range(8):
                    nc.sync.dma_start(
                        xs[:, e, t, :],
                        xT[128 * e:128 * (e + 1), CH * t:CH * (t + 1)])

            kT = persist.tile([128, NCH, CH], bf16, name="kT")
            qT = persist.tile([128, NCH, CH], bf16, name="qT")
            onesc_sb = const.tile([128, NCH * NSUB], bf16, name="onesc_sb")
            # V rows per k-subblock g: cols [h0 64 | ones | h1 64 | ones]
            vh = persist.tile([128, NCH * NSUB, 130], bf16, name="vh")
            if "onesc" in need:
                nc.sync.dma_start(onesc_sb[:], ones_c[:])
                nc.vector.tensor_copy(vh[:, :, 64], onesc_sb[:])
                nc.vector.tensor_copy(vh[:, :, 129], onesc_sb[:])

            cc_in = dram.tile([NCH, 2, 64, CH], bf16, name="cc_in")
            cc_out = dram.tile([NCH, 2, 64, CH], bf16, name="cc_out")

            # ---- phase 1: QKV projections + V transposes -------------------
            with tc.tile_pool(name="ps_qkv", bufs=2, space="PSUM") as ps_qkv, \
                 tc.tile_pool(name="ps_vt", bufs=2, space="PSUM") as ps_vt, \
                 tc.tile_pool(name="vstage", bufs=2) as vstage:
                kinds = ((wq_sb, bq_sb, 0.125, "q"),
                         (wk_sb, bk_sb, 1.0, "k"),
                         (wv_sb, bv_sb, 1.0, "v"))
                if stage == "qkv":
                    kinds = kinds[:1]
                elif stage == "vh":
                    kinds = kinds[2:]
                for t in range(NCH):
                    for wsb, bias, scale, kind in kinds:
                        ps = ps_qkv.tile([128, CH], f32, name="psqkv",
                                         tag="psqkv")
                        for e in range(8):
                            nc.tensor.matmul(ps[:], wsb[:, e, :],
                                             xs[:, e, t, :],
                                             start=(e == 0), stop=(e == 7))
                        if kind == "q":
                            nc.vector.tensor_scalar(
                                qT[:, t, :], ps[:], scale, bias[:],
                                ALU.mult, ALU.add)
                        elif kind == "k":
                            nc.vector.tensor_scalar_add(kT[:, t, :], ps[:],
                                                        bias[:])
                        else:
                            vts = vstage.tile([128, CH], f32r, name="vts",
                                              tag="vts")
                            nc.vector.tensor_scalar_add(vts[:], ps[:],
                                                        bias[:])
                            for s in range(NSUB):
                                tv = ps_vt.tile([128, 128], f32r, name="tv",
                                                tag="tv")
                                nc.tensor.transpose(
                                    tv[:], vts[:, 128 * s:128 * (s + 1)],
                                    eye_sb[:])
                                g = NSUB * t + s
                                nc.vector.tensor_copy(vh[:, g, 0:64],
                                                      tv[:, 0:64].bitcast(f32))
                                nc.vector.tensor_copy(vh[:, g, 65:129],
                                                      tv[:, 64:128].bitcast(f32))

            if stage == "qkv":
                with tc.tile_pool(name="dmp", bufs=2) as dmp:
                    for t in range(NCH):
                        dtile = dmp.tile([128, CH], bf16, name="dt",
                                         tag="dt")
                        nc.vector.tensor_copy(dtile[:], qT[:, t, :])
                        nc.sync.dma_start(y[:, CH * t:CH * (t + 1)],
                                          dtile[:])
                nc.compile()
                return nc
            if stage == "vh":
                with tc.tile_pool(name="dmp", bufs=2) as dmp:
                    for g in range(32):
                        dtile = dmp.tile([128, 130], bf16, name="dt",
                                         tag="dt")
                        nc.vector.tensor_copy(dtile[:], vh[:, g, :])
                        nc.sync.dma_start(
                            y[:, 130 * g:130 * (g + 1)], dtile[:])
                nc.compile()
                return nc

            if stage in ("full", "noa2a"):
                # wo load overlaps the attention phase
                for r in range(8):
                    nc.sync.dma_start(
                        wo_sb[:, r, :],
                        woT.rearrange("(p r) e -> r p e", p=8)[:, r, :])

            # ---- phase 2: attention ---------------------------------------
            # scores S^T = K Q^T per 128-row k-subblock; two subblocks share a
            # [128,1024] psum tile so one ACT exp covers both; causal mask is
            # an accumulated matmul stepU.T @ negI on the diagonal subblock.
            with tc.tile_pool(name="ps_s", bufs=2, space="PSUM") as ps_s, \
                 tc.tile_pool(name="ps_a", bufs=2, space="PSUM") as ps_a, \
                 tc.tile_pool(name="ps_bc", bufs=2, space="PSUM") as ps_bc, \
                 tc.tile_pool(name="ppool", bufs=6) as ppool, \
                 tc.tile_pool(name="araw", bufs=3) as araw, \
                 tc.tile_pool(name="npool", bufs=4) as npool:

                def emit_scores(t, kc, h):
                    """Scores+exp for chunk pair (q-chunk t, k-chunk kc), head
                    h. Returns (pT tiles, q0s) for the two tile-halves."""
                    pb = 64 * h
                    diag = kc == t
                    out = []
                    for half in range(2):
                        sps = ps_s.tile([128, 2 * CH], f32, name="sps",
                                        tag="sps")
                        pT = ppool.tile([128, 2 * CH], bf16, name="pT",
                                        tag="pT")
                        q0s = []
                        for i in range(2):
                            s = 2 * half + i
                            q0 = 128 * s if diag else 0
                            nc.tensor.matmul(
                                sps[:, CH * i + q0:CH * (i + 1)],
                                kT[pb:pb + 64, kc, 128 * s:128 * (s + 1)],
                                qT[pb:pb + 64, t, q0:CH],
                                start=True, stop=not diag)
                            if diag:
                                nc.tensor.matmul(
                                    sps[:, CH * i + q0:CH * i + q0 + 128],
                                    stepu_sb[:], negi_sb[:],
                                    start=False, stop=True)
                            q0s.append(q0)
                        nc.scalar.activation(pT[:, q0s[0]:2 * CH],
                                             sps[:, q0s[0]:2 * CH], AF.Exp)
                        out.append((pT, q0s))
                    return out

                def emit_av(t, kc, h, av, halves):
                    diag = kc == t
                    b0 = CPB * (t // CPB)
                    for half in range(2):
                        pT, q0s = halves[half]
                        for i in range(2):
                            s = 2 * half + i
                            q0 = q0s[i]
                            g = NSUB * kc + s
                            nc.tensor.matmul(
                                av[:, q0:CH], vh[:, g, 65 * h:65 * (h + 1)],
                                pT[:, CH * i + q0:CH * (i + 1)],
                                start=(kc == b0 and s == 0),
                                stop=(diag and s == NSUB - 1))

                def emit_norm_pre(t, h, av):
                    """Stage raw attention output + start the recip chain;
                    frees the av psum tile quickly."""
                    # av rows: 0:64 = head features, 64 = denominator
                    ar = araw.tile([65, CH], f32, name="ar", tag="ar")
                    nc.vector.tensor_copy(ar[:], av[:])
                    # custom-DVE op requires base partition 0: stage the
                    # denominator row into its own tile first
                    dn = npool.tile([1, CH], f32, name="dn", tag="dn")
                    nc.vector.tensor_copy(dn[:], ar[64:65, :])
                    rec = npool.tile([1, CH], f32, name="rec", tag="rec")
                    nc.vector.reciprocal_approx_fast(rec[:], dn[:])
                    recb = npool.tile([1, CH], bf16, name="recb", tag="recb")
                    nc.vector.tensor_copy(recb[:], rec[:])
                    return (t, h, ar, recb)

                def emit_norm_post(pending):
                    """Broadcast 1/den over 64 partitions (PE) and scale."""
                    t, h, ar, recb = pending
                    bc = ps_bc.tile([64, CH], f32, name="bc", tag="bc")
                    nc.tensor.matmul(bc[:], onesr_sb[:], recb[:],
                                     start=True, stop=True)
                    an = npool.tile([64, CH], bf16, name="an", tag="an")
                    nc.vector.tensor_mul(an[:], ar[0:64, :], bc[:])
                    if stage == "attn":
                        nc.sync.dma_start(y[t, h, :, :], an[:])
                    else:
                        nc.sync.dma_start(cc_in[t, h, :, :], an[:])

                pending_norms = []
                for t in range(NCH):
                    b0 = CPB * (t // CPB)
                    avs = [ps_a.tile([65, CH], f32, name=f"av{h}", tag="av")
                           for h in range(2)]
                    prev = None
                    for kc in range(b0, t + 1):
                        cur = (emit_scores(t, kc, 0), emit_scores(t, kc, 1))
                        if prev is None:
                            # previous chunk's norm tail overlaps these scores
                            for p in pending_norms:
                                emit_norm_post(p)
                            pending_norms = []
                        else:
                            pkc, ph = prev
                            emit_av(t, pkc, 0, avs[0], ph[0])
                            emit_av(t, pkc, 1, avs[1], ph[1])
                        prev = (kc, cur)
                    pkc, ph = prev
                    emit_av(t, pkc, 0, avs[0], ph[0])
                    emit_av(t, pkc, 1, avs[1], ph[1])
                    pending_norms = [emit_norm_pre(t, 0, avs[0]),
                                     emit_norm_pre(t, 1, avs[1])]
                for p in pending_norms:
                    emit_norm_post(p)

                if stage == "attn":
                    nc.compile()
                    return nc

                if stage == "noa2a":
                    for t in range(NCH):
                        for h in range(2):
                            nc.sync.dma_start(cc_out[t, h, :, :],
                                              cc_in[t, h, :, :])
                else:
                    nc.gpsimd.collective_compute(
                        "AllToAll", ALU.bypass,
                        ins=[cc_in.opt()], outs=[cc_out.opt()],
                        replica_groups=[list(range(NCORE))])

            # ---- phase 3: output projection on this core's token block -----
            with tc.tile_pool(name="ps_y", bufs=2, space="PSUM") as ps_y, \
                 tc.tile_pool(name="atb", bufs=1) as atb, \
                 tc.tile_pool(name="ystage", bufs=2) as ystage:
                aTb = atb.tile([128, 8, CH], bf16, name="aTb")
                for p in range(8):
                    for h in range(2):
                        nc.sync.dma_start(aTb[64 * h:64 * (h + 1), p, :],
                                          cc_out[p, h, :, :])
                for m in range(NSUB):
                    yps = ps_y.tile([128, E], f32, name="yps", tag="yps")
                    for half in range(2):
                        for p in range(8):
                            nc.tensor.matmul(
                                yps[:, CH * half:CH * (half + 1)],
                                aTb[:, p, 128 * m:128 * (m + 1)],
                                wo_sb[:, p, CH * half:CH * (half + 1)],
                                start=(p == 0), stop=(p == 7))
                    ysb = ystage.tile([128, E], f32, name="ysb", tag="ysb")
                    nc.vector.tensor_add(ysb[:], yps[:], bo_sb[:])
                    nc.sync.dma_start(y[128 * m:128 * (m + 1), :], ysb[:])
    nc.compile()
    return nc


def _prep_in_maps(embd_q, Wq, bq, Wk, bk, Wv, bv, Wo, bo):
    bf = ml_dtypes.bfloat16
    x = embd_q.reshape(NTOK, E).astype(np.float32)
    xT = np.ascontiguousarray(x.T.astype(bf))
    eye = np.eye(128, dtype=np.float32)
    ones_c = np.ones((128, NCH * NSUB), dtype=bf)
    r = np.arange(128)
    # stepu[p, k] = 1 for p < k ; negi = -1e30 * I
    stepu = np.ascontiguousarray((r[:, None] < r[None, :]).astype(bf))
    negi = np.ascontiguousarray((np.eye(128) * NEG).astype(bf))
    ones_r = np.ones((1, 64), dtype=bf)
    bo_b = np.ascontiguousarray(
        np.broadcast_to(bo.astype(np.float32), (128, E)))
    woT = np.ascontiguousarray(Wo.astype(np.float32).T.astype(bf))
    in_maps = []
    for c in range(NCORE):
        sl = slice(128 * c, 128 * (c + 1))
        in_maps.append({
            "xT": xT,
            "wqT": np.ascontiguousarray(Wq[sl].astype(np.float32).T.astype(bf)),
            "wkT": np.ascontiguousarray(Wk[sl].astype(np.float32).T.astype(bf)),
            "wvT": np.ascontiguousarray(Wv[sl].astype(np.float32).T.astype(bf)),
            "woT": woT,
            "bqs": np.ascontiguousarray(
                (bq[sl] * 0.125).reshape(128, 1), dtype=np.float32),
            "bks": np.ascontiguousarray(bk[sl].reshape(128, 1),
                                        dtype=np.float32),
            "bvs": np.ascontiguousarray(bv[sl].reshape(128, 1),
                                        dtype=np.float32),
            "bo_b": bo_b,
            "eye": eye,
            "ones_c": ones_c,
            "stepu": stepu,
            "negi": negi,
            "ones_r": ones_r,
        })
    return in_maps


def kernel(embd_q, Wq, bq, Wk, bk, Wv, bv, Wo, bo, _trace=False,
           _stage="full"):
    if _stage not in _nc_cache:
        _nc_cache[_stage] = build_nc(_stage)
    in_maps = _prep_in_maps(np.asarray(embd_q), np.asarray(Wq), np.asarray(bq),
                            np.asarray(Wk), np.asarray(bk), np.asarray(Wv),
                            np.asarray(bv), np.asarray(Wo), np.asarray(bo))
    import os
    tc_env = os.environ.get("TRACE_CORES")
    res = run_bass_kernel_spmd(
        _nc_cache[_stage], in_maps, list(range(NCORE)), trace=_trace,
        trace_cores=(list(range(NCORE)) if tc_env else None))
    kernel.last_results = res
    if _stage not in ("full", "noa2a"):
        return None
    out = np.concatenate(
        [res.results[c]["y"] for c in range(NCORE)], axis=0)
    out = out.reshape(B, T, E)
    return out
